# revision 11
# baseline (speedup 1.0000x reference)
"""AngleClassificationLoss Trainium2 kernel.

loss = BCE(probs[:,0], smooth_labels(gt_u)) + BCE(probs[:,1], smooth_labels(gt_r))

Decomposition used here (exact up to f32 rounding):
    BCE * N = -( sum(log(1-p))  +  sum_b (1/Z_b) * sum_window u*v*(log p - log(1-p)) )
where the smoothed label of example b is a separable sigma=1 gaussian centered
at (theta_bin, phi_bin), cropped to the grid and renormalized by Z_b. In f32
the gaussian is identically zero beyond ~13 bins from the center, so a 33x33
window captures the label term exactly at f32 precision.

Sharding: pure data parallel over batch (1024 -> 8 cores x 128 examples).
Each core returns per-partition partial sums [128, 2]; the host reduces in f64.
"""

import numpy as np

P = 128                     # examples per core (batch shard), also SBUF partitions
N_CORES = 8
N_THETA, N_PHI = 180, 360
CH = N_THETA * N_PHI        # 64800 elements per channel grid
EX = 2 * CH                 # 129600 elements per example
W = 33                      # label window size (center +/- 16)
HALF = 16
F = 8100                    # main-pass tile free size (divides EX)
NT = EX // F                # 16 tiles per core
N_MEAN = 1024 * CH          # per-channel mean divisor in the reference
RAD2BIN = 57.29577951308232  # 180/pi
PI = 3.141592653589793

_CACHE = {}


def _build_nc(dbg=False):
    import concourse.bacc as bacc
    import concourse.tile as tile
    from concourse import bass, mybir

    f32 = mybir.dt.float32
    i32 = mybir.dt.int32
    AF = mybir.ActivationFunctionType
    OP = mybir.AluOpType
    AX = mybir.AxisListType

    nc = bacc.Bacc(
        "TRN2",
        target_bir_lowering=False,
        debug=False,
        enable_asserts=False,
        num_devices=N_CORES,
    )
    probs_t = nc.dram_tensor("probs", [P, 2, N_THETA, N_PHI], f32, kind="ExternalInput")
    gt_t = [
        nc.dram_tensor("gt_u", [P, 3], f32, kind="ExternalInput"),
        nc.dram_tensor("gt_r", [P, 3], f32, kind="ExternalInput"),
    ]
    out_t = nc.dram_tensor("out", [P, 2], f32, kind="ExternalOutput")
    if dbg:
        dbg_idx = [nc.dram_tensor(f"dbg_idx{c}", [P, W], i32, kind="ExternalOutput")
                   for c in (0, 1)]
        dbg_win = [nc.dram_tensor(f"dbg_win{c}", [P, W * W], f32,
                                  kind="ExternalOutput") for c in (0, 1)]
        dbg_scr = [nc.dram_tensor(f"dbg_scr{c}", [P, 40], f32,
                                  kind="ExternalOutput") for c in (0, 1)]

    probs2d = probs_t.ap().rearrange("b c t p -> b (c t p)")  # [128, 129600]
    probs1d = probs_t.ap().flatten().unsqueeze(1)             # [TOTAL, 1]

    def bcast_mid(ap2d, n):
        # [P, W] -> [P, n, W] with step-0 middle dim (free-dim broadcast)
        return bass.AP(
            tensor=ap2d.tensor,
            offset=ap2d.offset,
            ap=[list(ap2d.ap[0]), [0, n], list(ap2d.ap[1])],
        )

    with tile.TileContext(nc) as tc:
        with (
            tc.tile_pool(name="main", bufs=4) as mainp,
            tc.tile_pool(name="winp", bufs=1) as winp,
            tc.tile_pool(name="small", bufs=1) as small,
        ):
            # ---------- shared constants ----------
            jio_i = small.tile([P, W], i32)
            nc.gpsimd.iota(jio_i[:], pattern=[[1, W]], base=0, channel_multiplier=0)
            jio_f = small.tile([P, W], f32)
            nc.vector.tensor_copy(out=jio_f[:], in_=jio_i[:])
            rowio = small.tile([P, W], i32)     # r*360
            nc.gpsimd.iota(rowio[:], pattern=[[N_PHI, W]], base=0, channel_multiplier=0)
            pio = small.tile([P, 1], i32)       # partition*129600
            nc.gpsimd.iota(pio[:], pattern=[[0, 1]], base=0, channel_multiplier=EX)

            s2tot = small.tile([P, 1], f32)
            nc.vector.memset(s2tot[:], 0.0)

            # ---------- per-channel window (label) term ----------
            for c in (0, 1):
                scr = small.tile([P, 40], f32, tag=f"scr{c}")
                cols = iter(range(40))

                def col(it=cols):
                    i = next(it)
                    return scr[:, i : i + 1]

                g = small.tile([P, 3], f32, tag=f"gt{c}")
                nc.sync.dma_start(out=g[:], in_=gt_t[c].ap())
                gx, gy, gz = g[:, 0:1], g[:, 1:2], g[:, 2:3]

                # ---- theta bin: theta = arccos(clip(z,-1,1)) via half-angle arctan
                zc = col()
                nc.vector.tensor_scalar(out=zc, in0=gz, scalar1=1.0, scalar2=-1.0,
                                        op0=OP.min, op1=OP.max)
                z2 = col()
                nc.scalar.activation(out=z2, in_=zc, func=AF.Square)
                rxy = col()   # sqrt(1-z^2)
                nc.scalar.activation(out=rxy, in_=z2, func=AF.Sqrt, scale=-1.0, bias=1.0)
                az = col()
                nc.scalar.activation(out=az, in_=zc, func=AF.Abs)
                den = col()
                nc.vector.tensor_scalar_add(out=den, in0=az, scalar1=1.0)
                rden = col()
                nc.vector.reciprocal(out=rden, in_=den)
                arg = col()
                nc.vector.tensor_tensor(out=arg, in0=rxy, in1=rden, op=OP.mult)
                at = col()
                nc.scalar.activation(out=at, in_=arg, func=AF.Arctan)
                m = col()
                nc.vector.tensor_scalar(out=m, in0=zc, scalar1=0.0, scalar2=None,
                                        op0=OP.is_ge)
                c1 = col()    # (1-m)*pi
                nc.vector.tensor_scalar(out=c1, in0=m, scalar1=-PI, scalar2=PI,
                                        op0=OP.mult, op1=OP.add)
                c2 = col()    # 4m-2
                nc.vector.tensor_scalar(out=c2, in0=m, scalar1=4.0, scalar2=-2.0,
                                        op0=OP.mult, op1=OP.add)
                th = col()
                nc.vector.tensor_tensor(out=th, in0=c2, in1=at, op=OP.mult)
                nc.vector.tensor_tensor(out=th, in0=th, in1=c1, op=OP.add)
                tf = col()
                nc.vector.tensor_scalar_mul(out=tf, in0=th, scalar1=RAD2BIN)
                nc.vector.tensor_scalar(out=tf, in0=tf, scalar1=0.0, scalar2=179.0,
                                        op0=OP.max, op1=OP.min)
                tstar_i = small.tile([P, 1], i32, tag=f"ti{c}")
                nc.vector.tensor_copy(out=tstar_i[:], in_=tf)
                tstar = col()
                nc.vector.tensor_copy(out=tstar, in_=tstar_i[:])

                # ---- phi bin: atan2(y, x) in [0, 2pi) via half-angle arctan
                x2 = col()
                nc.scalar.activation(out=x2, in_=gx, func=AF.Square)
                y2 = col()
                nc.scalar.activation(out=y2, in_=gy, func=AF.Square)
                nc.vector.tensor_tensor(out=x2, in0=x2, in1=y2, op=OP.add)
                rr = col()
                nc.scalar.activation(out=rr, in_=x2, func=AF.Sqrt)
                ax = col()
                nc.scalar.activation(out=ax, in_=gx, func=AF.Abs)
                ay = col()
                nc.scalar.activation(out=ay, in_=gy, func=AF.Abs)
                nc.vector.tensor_tensor(out=ax, in0=rr, in1=ax, op=OP.add)
                nc.vector.tensor_scalar_add(out=ax, in0=ax, scalar1=1e-30)
                nc.vector.reciprocal(out=ax, in_=ax)
                nc.vector.tensor_tensor(out=ay, in0=ay, in1=ax, op=OP.mult)
                a2 = col()
                nc.scalar.activation(out=a2, in_=ay, func=AF.Arctan)
                mx = col()
                nc.vector.tensor_scalar(out=mx, in0=gx, scalar1=0.0, scalar2=None,
                                        op0=OP.is_ge)
                my = col()
                nc.vector.tensor_scalar(out=my, in0=gy, scalar1=0.0, scalar2=None,
                                        op0=OP.is_ge)
                d1 = col()    # 4mx-2
                nc.vector.tensor_scalar(out=d1, in0=mx, scalar1=4.0, scalar2=-2.0,
                                        op0=OP.mult, op1=OP.add)
                nc.vector.tensor_tensor(out=d1, in0=d1, in1=a2, op=OP.mult)
                d2 = col()    # (1-mx)*pi
                nc.vector.tensor_scalar(out=d2, in0=mx, scalar1=-PI, scalar2=PI,
                                        op0=OP.mult, op1=OP.add)
                nc.vector.tensor_tensor(out=d1, in0=d1, in1=d2, op=OP.add)  # psi=|phi|
                sy = col()    # 2my-1
                nc.vector.tensor_scalar(out=sy, in0=my, scalar1=2.0, scalar2=-1.0,
                                        op0=OP.mult, op1=OP.add)
                ph = col()
                nc.vector.tensor_tensor(out=ph, in0=d1, in1=sy, op=OP.mult)
                neg = col()
                nc.vector.tensor_scalar(out=neg, in0=ph, scalar1=0.0, scalar2=None,
                                        op0=OP.is_lt)
                nc.vector.tensor_scalar_mul(out=neg, in0=neg, scalar1=2.0 * PI)
                nc.vector.tensor_tensor(out=ph, in0=ph, in1=neg, op=OP.add)
                pf = col()
                nc.vector.tensor_scalar_mul(out=pf, in0=ph, scalar1=RAD2BIN)
                nc.vector.tensor_scalar(out=pf, in0=pf, scalar1=0.0, scalar2=359.0,
                                        op0=OP.max, op1=OP.min)
                pstar_i = small.tile([P, 1], i32, tag=f"pi{c}")
                nc.vector.tensor_copy(out=pstar_i[:], in_=pf)
                pstar = col()
                nc.vector.tensor_copy(out=pstar, in_=pstar_i[:])

                # ---- window geometry
                t0 = col()
                nc.vector.tensor_scalar(out=t0, in0=tstar, scalar1=float(HALF),
                                        scalar2=None, op0=OP.subtract)
                nc.vector.tensor_scalar(out=t0, in0=t0, scalar1=0.0,
                                        scalar2=float(N_THETA - W), op0=OP.max, op1=OP.min)
                nst = col()   # -(tstar - t0)
                nc.vector.tensor_tensor(out=nst, in0=t0, in1=tstar, op=OP.subtract)
                p0 = col()
                nc.vector.tensor_scalar(out=p0, in0=pstar, scalar1=float(HALF),
                                        scalar2=None, op0=OP.subtract)
                nc.vector.tensor_scalar(out=p0, in0=p0, scalar1=0.0,
                                        scalar2=float(N_PHI - W), op0=OP.max, op1=OP.min)
                nsp = col()   # -(pstar - p0)
                nc.vector.tensor_tensor(out=nsp, in0=p0, in1=pstar, op=OP.subtract)

                base = col()  # t0*360 + p0 + c*64800
                nc.vector.tensor_scalar(out=base, in0=t0, scalar1=float(N_PHI),
                                        scalar2=float(c * CH), op0=OP.mult, op1=OP.add)
                nc.vector.tensor_tensor(out=base, in0=base, in1=p0, op=OP.add)
                base_i = small.tile([P, 1], i32, tag=f"bi{c}")
                nc.vector.tensor_copy(out=base_i[:], in_=base)
                nc.vector.tensor_tensor(out=base_i[:], in0=base_i[:], in1=pio[:],
                                        op=OP.add)
                idx = small.tile([P, W], i32, tag=f"idx{c}")
                nc.vector.tensor_tensor(out=idx[:], in0=rowio[:],
                                        in1=base_i[:, 0:1].to_broadcast([P, W]),
                                        op=OP.add)

                # ---- separable gaussian weights (f32 underflow crops the tails)
                vv = small.tile([P, W], f32, tag=f"vv{c}")
                nc.scalar.activation(out=vv[:], in_=jio_f[:], func=AF.Square, bias=nsp)
                nc.scalar.activation(out=vv[:], in_=vv[:], func=AF.Exp, scale=-0.5)
                uu = small.tile([P, W], f32, tag=f"uu{c}")
                nc.scalar.activation(out=uu[:], in_=jio_f[:], func=AF.Square, bias=nst)
                nc.scalar.activation(out=uu[:], in_=uu[:], func=AF.Exp, scale=-0.5)
                zu = col()
                nc.vector.tensor_reduce(out=zu, in_=uu[:], axis=AX.X, op=OP.add)
                zv = col()
                nc.vector.tensor_reduce(out=zv, in_=vv[:], axis=AX.X, op=OP.add)
                nc.vector.tensor_tensor(out=zu, in0=zu, in1=zv, op=OP.mult)
                rz = col()
                nc.vector.reciprocal(out=rz, in_=zu)

                # ---- gather the 33x33 window of probs around each center
                # one indirect DMA per window row: [P,1] offsets (proven shape)
                win = winp.tile([P, W, W], f32, tag=f"win{c}")
                for r in range(W):
                    nc.gpsimd.indirect_dma_start(
                        out=win[:, r, :],
                        out_offset=None,
                        in_=probs1d,
                        in_offset=bass.IndirectOffsetOnAxis(
                            ap=idx[:, r : r + 1], axis=0
                        ),
                    )
                win2 = win[:].rearrange("p a b -> p (a b)")
                if dbg:
                    nc.sync.dma_start(out=dbg_idx[c].ap(), in_=idx[:])
                    nc.sync.dma_start(out=dbg_win[c].ap(), in_=win2)
                lq = winp.tile([P, W * W], f32, tag=f"lq{c}")
                nc.scalar.activation(out=lq[:], in_=win2, func=AF.Ln, scale=-1.0,
                                     bias=1.0)
                nc.scalar.activation(out=win2, in_=win2, func=AF.Ln)
                nc.vector.tensor_tensor(out=lq[:], in0=win2, in1=lq[:], op=OP.subtract)
                # multiply by v along phi and reduce
                lq3 = lq[:].rearrange("p (a b) -> p a b", a=W)
                nc.vector.tensor_tensor(out=lq3, in0=lq3, in1=bcast_mid(vv[:], W),
                                        op=OP.mult)
                rsum = small.tile([P, W], f32, tag=f"rs{c}")
                nc.vector.tensor_reduce(out=rsum[:], in_=lq3, axis=AX.X, op=OP.add)
                nc.vector.tensor_tensor(out=rsum[:], in0=rsum[:], in1=uu[:], op=OP.mult)
                s2c = col()
                nc.vector.tensor_reduce(out=s2c, in_=rsum[:], axis=AX.X, op=OP.add)
                nc.vector.tensor_tensor(out=s2c, in0=s2c, in1=rz, op=OP.mult)
                nc.vector.tensor_tensor(out=s2tot[:], in0=s2tot[:], in1=s2c, op=OP.add)
                if dbg:
                    nc.sync.dma_start(out=dbg_scr[c].ap(), in_=scr[:])

            # ---------- dense pass: sum log(1-p) over everything ----------
            stats = small.tile([P, NT], f32)
            for i in range(NT):
                mt = mainp.tile([P, F], f32, tag="mt")
                nc.sync.dma_start(out=mt[:], in_=probs2d[:, i * F : (i + 1) * F])
                nc.scalar.activation(out=mt[:], in_=mt[:], func=AF.Ln, scale=-1.0,
                                     bias=1.0, accum_out=stats[:, i : i + 1])

            # ---------- assemble output ----------
            outt = small.tile([P, 2], f32)
            nc.vector.tensor_reduce(out=outt[:, 0:1], in_=stats[:], axis=AX.X,
                                    op=OP.add)
            nc.vector.tensor_copy(out=outt[:, 1:2], in_=s2tot[:])
            nc.sync.dma_start(out=out_t.ap(), in_=outt[:])

    nc.compile()
    return nc


def _get_nc():
    if "nc" not in _CACHE:
        _CACHE["nc"] = _build_nc()
    return _CACHE["nc"]


def _run_on_hw(in_maps, trace=False, **kw):
    from concourse.bass_utils import run_bass_kernel_spmd

    return run_bass_kernel_spmd(_get_nc(), in_maps, core_ids=list(range(N_CORES)),
                                trace=trace, **kw)


def _make_in_maps(probs, gt_u, gt_r):
    probs = np.ascontiguousarray(np.asarray(probs, dtype=np.float32))
    gt_u = np.ascontiguousarray(np.asarray(gt_u, dtype=np.float32))
    gt_r = np.ascontiguousarray(np.asarray(gt_r, dtype=np.float32))
    assert probs.shape == (N_CORES * P, 2, N_THETA, N_PHI)
    return [
        {
            "probs": probs[i * P : (i + 1) * P],
            "gt_u": gt_u[i * P : (i + 1) * P],
            "gt_r": gt_r[i * P : (i + 1) * P],
        }
        for i in range(N_CORES)
    ]


def _combine(results):
    tot = np.float64(0.0)
    for r in results:
        tot += r["out"].astype(np.float64).sum()
    return np.float32(-(tot / N_MEAN))


def kernel(probs, gt_u, gt_r):
    res = _run_on_hw(_make_in_maps(probs, gt_u, gt_r))
    return _combine(res.results)
